# revision 10
# baseline (speedup 1.0000x reference)
"""Trainium2 Bass kernel for nn_BasicConvolutionBlock (sparse conv + BN + ReLU).

Math (per reference):
    conv[n] = sum_k feats[nbr_idx[n, k]] @ W[k]       # [N, 96], k = 0..26
    y = (conv - mean) * rsqrt(var + eps) * gamma + beta ; relu(y)

Distribution: voxel (N) dimension sharded across 8 NeuronCores; feats table
and weights replicated per core.

Two NEFFs (the AllReduce-in-kernel path is unstable under the axon PJRT
bridge, so per-core BN partial sums are combined on the host -- 768 B of
float math -- between the two device passes):

  pass 1 (per core, per 512-voxel tile):
    - 108 indirect DMA gathers (128 rows x 96 f32) -> SBUF [128, 108, 96]
    - per offset k: 4 PE transposes [128v, 96c] -> PSUM [96, 512v]
      DVE copy PSUM -> SBUF rounding to f32r
      accumulating f32r matmul W_k.T @ G_k.T -> PSUM [96, 512]
    - BN partial sum/sumsq via ACT accum; conv staged to DRAM channel-major
  pass 2: y = relu(conv * scale + shift) (fused ACT op), PE transpose back
    to row-major, store.
"""
import numpy as np

import concourse.bass as bass
import concourse.bacc as bacc
import concourse.tile as tile
import concourse.mybir as mybir
from concourse.masks import make_identity

F32 = mybir.dt.float32
F32R = mybir.dt.float32r
I32 = mybir.dt.int32
AF = mybir.ActivationFunctionType

N_TOTAL = 262144
C = 96
KVOL = 27
N_CORES = 8
N_PER_CORE = N_TOTAL // N_CORES      # 32768
TILE_V = 512                         # voxels per tile
BLOCKS = TILE_V // 128               # 4
J = KVOL * BLOCKS                    # 108 gathers per tile
N_TILES = N_PER_CORE // TILE_V       # 64
BN_EPS = 1e-5

_cache = {}


# --------------------------------------------------------------------------
# graph builders
# --------------------------------------------------------------------------
def build_pass1(n_tiles=N_TILES):
    nc = bacc.Bacc("TRN2", target_bir_lowering=False, debug=False,
                   num_devices=N_CORES)

    feats = nc.dram_tensor("feats", [N_TOTAL, C], F32, kind="ExternalInput")
    # host-arranged: idx_arr[t, p, k*4+b] = nbr[t*512 + b*128 + p, k]
    idx_arr = nc.dram_tensor("idx_arr", [n_tiles, 128, J], I32,
                             kind="ExternalInput")
    W = nc.dram_tensor("W", [KVOL, C, C], F32, kind="ExternalInput")
    convT = nc.dram_tensor("convT", [C, n_tiles * TILE_V], F32,
                           kind="ExternalOutput")
    stats = nc.dram_tensor("stats", [C, 2], F32, kind="ExternalOutput")

    with tile.TileContext(nc) as tc:
        with (
            tc.tile_pool(name="const", bufs=1) as cpool,
            tc.tile_pool(name="gp", bufs=2) as gpool,
            tc.tile_pool(name="rp", bufs=3) as rpool,
            tc.tile_pool(name="sp", bufs=2) as spool,
            tc.tile_pool(name="psA", bufs=2, space="PSUM") as psA,
            tc.tile_pool(name="psB", bufs=2, space="PSUM") as psB,
        ):
            ident = cpool.tile([128, 128], F32, tag="ident")
            make_identity(nc, ident[:])
            w_sb = cpool.tile([C, KVOL, C], F32, tag="w")
            nc.sync.dma_start(w_sb[:], W[:].rearrange("k ci co -> ci k co"))
            w_sbr = cpool.tile([C, KVOL, C], F32R, tag="wr")
            nc.vector.tensor_copy(w_sbr[:], w_sb[:])

            sum_acc = cpool.tile([C, n_tiles], F32, tag="sum_acc")
            sq_acc = cpool.tile([C, n_tiles], F32, tag="sq_acc")

            for t in range(n_tiles):
                idx_sb = gpool.tile([128, J], I32, tag="idx")
                nc.sync.dma_start(idx_sb[:], idx_arr[t, :, :])

                g_sb = gpool.tile([128, J, C], F32, tag="g")
                for j in range(J):
                    nc.gpsimd.indirect_dma_start(
                        out=g_sb[:, j, :],
                        out_offset=None,
                        in_=feats[:],
                        in_offset=bass.IndirectOffsetOnAxis(
                            ap=idx_sb[:, j:j + 1], axis=0),
                    )

                out_ps = psA.tile([C, TILE_V], F32, tag="outp")
                for k in range(KVOL):
                    tp = psB.tile([C, TILE_V], F32, tag="tp")
                    for b in range(BLOCKS):
                        nc.tensor.transpose(
                            tp[:, b * 128:(b + 1) * 128],
                            g_sb[:, k * BLOCKS + b, :],
                            ident[:],
                        )
                    rhs = rpool.tile([C, TILE_V], F32R, tag="rhs")
                    nc.vector.tensor_copy(rhs[:], tp[:])
                    nc.tensor.matmul(
                        out_ps[:], w_sbr[:, k, :], rhs[:],
                        start=(k == 0), stop=(k == KVOL - 1),
                    )

                conv_sb = spool.tile([C, TILE_V], F32, tag="conv")
                nc.scalar.activation(
                    conv_sb[:], out_ps[:], AF.Identity,
                    accum_out=sum_acc[:, t:t + 1])
                sq_sb = spool.tile([C, TILE_V], F32, tag="sq")
                nc.scalar.activation(
                    sq_sb[:], conv_sb[:], AF.Square,
                    accum_out=sq_acc[:, t:t + 1])
                nc.sync.dma_start(
                    convT[:, t * TILE_V:(t + 1) * TILE_V], conv_sb[:])

            stats_sb = cpool.tile([C, 2], F32, tag="stats_sb")
            nc.vector.reduce_sum(
                stats_sb[:, 0:1], sum_acc[:], axis=mybir.AxisListType.X)
            nc.vector.reduce_sum(
                stats_sb[:, 1:2], sq_acc[:], axis=mybir.AxisListType.X)
            nc.sync.dma_start(stats[:], stats_sb[:])

    nc.finalize()
    return nc


def build_pass2(n_tiles=N_TILES):
    nc = bacc.Bacc("TRN2", target_bir_lowering=False, debug=False,
                   num_devices=N_CORES)
    convT = nc.dram_tensor("convT", [C, n_tiles * TILE_V], F32,
                           kind="ExternalInput")
    scale = nc.dram_tensor("scale", [C, 1], F32, kind="ExternalInput")
    shift = nc.dram_tensor("shift", [C, 1], F32, kind="ExternalInput")
    out = nc.dram_tensor("out", [n_tiles * TILE_V, C], F32,
                         kind="ExternalOutput")

    with tile.TileContext(nc) as tc:
        with (
            tc.tile_pool(name="const", bufs=1) as cpool,
            tc.tile_pool(name="sp", bufs=3) as spool,
            tc.tile_pool(name="ps", bufs=2, space="PSUM") as ps,
        ):
            ident = cpool.tile([C, C], F32, tag="ident")
            make_identity(nc, ident[:])
            scale_sb = cpool.tile([C, 1], F32, tag="scale")
            shift_sb = cpool.tile([C, 1], F32, tag="shift")
            nc.sync.dma_start(scale_sb[:], scale[:])
            nc.sync.dma_start(shift_sb[:], shift[:])

            for t in range(n_tiles):
                nsb = spool.tile([C, TILE_V], F32, tag="nsb")
                nc.sync.dma_start(
                    nsb[:], convT[:, t * TILE_V:(t + 1) * TILE_V])
                nrm = spool.tile([C, TILE_V], F32, tag="nrm")
                nc.scalar.activation(
                    nrm[:], nsb[:], AF.Relu,
                    bias=shift_sb[:], scale=scale_sb[:])
                op = ps.tile([128, BLOCKS * C], F32, tag="op")
                for b in range(BLOCKS):
                    nc.tensor.transpose(
                        op[:, b * C:(b + 1) * C],
                        nrm[:, b * 128:(b + 1) * 128],
                        ident[:],
                    )
                osb = spool.tile([128, BLOCKS * C], F32, tag="osb")
                nc.vector.tensor_copy(osb[:], op[:])
                nc.sync.dma_start(
                    out[t * TILE_V:(t + 1) * TILE_V, :].rearrange(
                        "(b p) c -> p b c", p=128),
                    osb[:].rearrange("p (b c) -> p b c", b=BLOCKS),
                )

    nc.finalize()
    return nc


# --------------------------------------------------------------------------
# reusable PJRT runner (keeps the compiled executable across calls)
# --------------------------------------------------------------------------
class _Runner:
    def __init__(self, nc, n_cores):
        import jax
        from jax.sharding import Mesh, PartitionSpec
        from jax.experimental.shard_map import shard_map
        from concourse import bass2jax

        bass2jax.install_neuronx_cc_hook()
        self.jax = jax
        self.n_cores = n_cores
        pname = nc.partition_id_tensor.name if nc.partition_id_tensor else None
        in_names, out_names, out_avals, zero_outs = [], [], [], []
        for alloc in nc.m.functions[0].allocations:
            if not isinstance(alloc, mybir.MemoryLocationSet):
                continue
            name = alloc.memorylocations[0].name
            if alloc.kind == "ExternalInput":
                if name != pname:
                    in_names.append(name)
            elif alloc.kind == "ExternalOutput":
                out_names.append(name)
                shape = tuple(alloc.tensor_shape)
                dtype = mybir.dt.np(alloc.dtype)
                out_avals.append(jax.core.ShapedArray(shape, dtype))
                zero_outs.append(np.zeros(shape, dtype))
        self.in_names, self.out_names = in_names, out_names
        self.out_avals, self.zero_outs = out_avals, zero_outs
        n_params = len(in_names)
        self.n_params = n_params
        all_in = list(in_names) + list(out_names)
        if pname is not None:
            all_in.append(pname)

        def _body(*args):
            operands = list(args)
            if pname is not None:
                operands.append(bass2jax.partition_id_tensor())
            outs = bass2jax._bass_exec_p.bind(
                *operands,
                out_avals=tuple(out_avals),
                in_names=tuple(all_in),
                out_names=tuple(out_names),
                lowering_input_output_aliases=(),
                sim_require_finite=True,
                sim_require_nnan=True,
                nc=nc,
            )
            return tuple(outs)

        devices = jax.devices()[:n_cores]
        mesh = Mesh(np.asarray(devices), ("core",))
        nspec = (PartitionSpec("core"),) * (n_params + len(out_names))
        self.fn = jax.jit(
            shard_map(_body, mesh=mesh, in_specs=nspec,
                      out_specs=(PartitionSpec("core"),) * len(out_names),
                      check_rep=False),
            keep_unused=True,
        )

    def run(self, in_maps):
        per_core = [[np.asarray(m[n]) for n in self.in_names]
                    for m in in_maps]
        args = [
            np.concatenate([per_core[c][i] for c in range(self.n_cores)],
                           axis=0)
            for i in range(self.n_params)
        ]
        args += [
            np.zeros((self.n_cores * z.shape[0], *z.shape[1:]), z.dtype)
            for z in self.zero_outs
        ]
        outs = self.fn(*args)
        self.jax.block_until_ready(outs)
        return [
            {
                n: np.asarray(outs[i]).reshape(
                    self.n_cores, *self.out_avals[i].shape)[c]
                for i, n in enumerate(self.out_names)
            }
            for c in range(self.n_cores)
        ]


# --------------------------------------------------------------------------
# host-side glue
# --------------------------------------------------------------------------
def _arrange_idx(nbr_shard: np.ndarray, n_tiles: int) -> np.ndarray:
    """[n_tiles*512, 27] -> [n_tiles, 128, 108] with
    idx_arr[t, p, k*4+b] = nbr[t*512 + b*128 + p, k]."""
    a = nbr_shard.reshape(n_tiles, BLOCKS, 128, KVOL)       # [t, b, p, k]
    a = a.transpose(0, 2, 3, 1)                             # [t, p, k, b]
    return np.ascontiguousarray(a.reshape(n_tiles, 128, J), dtype=np.int32)


def run_pipeline(feats, nbr, W, gamma, beta, n_tiles):
    n_total = n_tiles * TILE_V * N_CORES
    npc = n_tiles * TILE_V
    key = ("p1", n_tiles)
    if key not in _cache:
        _cache[key] = _Runner(build_pass1(n_tiles), N_CORES)
    r1 = _cache[key]
    key2 = ("p2", n_tiles)
    if key2 not in _cache:
        _cache[key2] = _Runner(build_pass2(n_tiles), N_CORES)
    r2 = _cache[key2]

    in1 = []
    for c in range(N_CORES):
        shard = nbr[c * npc:(c + 1) * npc]
        in1.append({
            "feats": feats,
            "idx_arr": _arrange_idx(shard, n_tiles),
            "W": W,
        })
    res1 = r1.run(in1)

    # combine BN stats on host (768 B per core)
    s = np.sum([r["stats"] for r in res1], axis=0, dtype=np.float64)
    mean = s[:, 0] / n_total
    var = s[:, 1] / n_total - mean * mean
    inv = 1.0 / np.sqrt(var + BN_EPS)
    scale = (gamma.astype(np.float64).reshape(C) * inv).astype(np.float32)
    shift = (beta.astype(np.float64).reshape(C)
             - mean * gamma.astype(np.float64).reshape(C) * inv
             ).astype(np.float32)

    in2 = [{
        "convT": res1[c]["convT"],
        "scale": scale.reshape(C, 1),
        "shift": shift.reshape(C, 1),
    } for c in range(N_CORES)]
    res2 = r2.run(in2)
    return np.concatenate([r["out"] for r in res2], axis=0)


def kernel(feats, nbr_idx, W, gamma, beta):
    feats = np.ascontiguousarray(feats, dtype=np.float32)
    W = np.ascontiguousarray(W, dtype=np.float32)
    nbr = np.asarray(nbr_idx)
    gamma = np.asarray(gamma, dtype=np.float32)
    beta = np.asarray(beta, dtype=np.float32)
    return run_pipeline(feats, nbr, W, gamma, beta, N_TILES)


# revision 13
# speedup vs baseline: 178.3673x; 178.3673x over previous
"""Trainium2 Bass kernel for nn_BasicConvolutionBlock (sparse conv + BN + ReLU).

Math (per reference):
    conv[n] = sum_k feats[nbr_idx[n, k]] @ W[k]       # [N, 96], k = 0..26
    y = (conv - mean) * rsqrt(var + eps) * gamma + beta ; relu(y)

Distribution: voxel (N) dimension sharded across 8 NeuronCores; feats table
and weights replicated per core.

Two NEFFs (the AllReduce-in-kernel path is unstable under the axon PJRT
bridge, so per-core BN partial sums are combined on the host -- 768 B of
float math -- between the two device passes):

  pass 1 (per core, per 512-voxel tile):
    - 108 indirect DMA gathers (128 rows x 96 f32) -> SBUF [128, 108, 96]
    - per offset k: 4 PE transposes [128v, 96c] -> PSUM [96, 512v]
      DVE copy PSUM -> SBUF rounding to f32r
      accumulating f32r matmul W_k.T @ G_k.T -> PSUM [96, 512]
    - BN partial sum/sumsq via ACT accum; conv staged to DRAM channel-major
  pass 2: y = relu(conv * scale + shift) (fused ACT op), PE transpose back
    to row-major, store.
"""
import numpy as np

import concourse.bass as bass
import concourse.bacc as bacc
import concourse.tile as tile
import concourse.mybir as mybir
from concourse.masks import make_identity

F32 = mybir.dt.float32
F32R = mybir.dt.float32r
I32 = mybir.dt.int32
AF = mybir.ActivationFunctionType

N_TOTAL = 262144
C = 96
KVOL = 27
N_CORES = 8
N_PER_CORE = N_TOTAL // N_CORES      # 32768
TILE_V = 512                         # voxels per tile
BLOCKS = TILE_V // 128               # 4
J = KVOL * BLOCKS                    # 108 gathers per tile
N_TILES = N_PER_CORE // TILE_V       # 64
BN_EPS = 1e-5

_cache = {}


# --------------------------------------------------------------------------
# graph builders
# --------------------------------------------------------------------------
def build_pass1(n_tiles=N_TILES):
    nc = bacc.Bacc("TRN2", target_bir_lowering=False, debug=False,
                   num_devices=N_CORES)

    feats = nc.dram_tensor("feats", [N_TOTAL, C], F32, kind="ExternalInput")
    # host-arranged: idx_arr[t, p, k*4+b] = nbr[t*512 + b*128 + p, k]
    idx_arr = nc.dram_tensor("idx_arr", [n_tiles, 128, J], I32,
                             kind="ExternalInput")
    W = nc.dram_tensor("W", [KVOL, C, C], F32, kind="ExternalInput")
    convT = nc.dram_tensor("convT", [C, n_tiles * TILE_V], F32,
                           kind="ExternalOutput")
    stats = nc.dram_tensor("stats", [C, 2], F32, kind="ExternalOutput")

    with tile.TileContext(nc) as tc:
        with (
            tc.tile_pool(name="const", bufs=1) as cpool,
            tc.tile_pool(name="gp", bufs=2) as gpool,
            tc.tile_pool(name="rp", bufs=3) as rpool,
            tc.tile_pool(name="sp", bufs=2) as spool,
            tc.tile_pool(name="psA", bufs=2, space="PSUM") as psA,
            tc.tile_pool(name="psB", bufs=2, space="PSUM") as psB,
        ):
            ident = cpool.tile([128, 128], F32, tag="ident")
            make_identity(nc, ident[:])
            w_sb = cpool.tile([C, KVOL, C], F32, tag="w")
            nc.sync.dma_start(w_sb[:], W[:].rearrange("k ci co -> ci k co"))
            w_sbr = cpool.tile([C, KVOL, C], F32R, tag="wr")
            nc.vector.tensor_copy(w_sbr[:], w_sb[:])

            sum_acc = cpool.tile([C, n_tiles], F32, tag="sum_acc")
            sq_acc = cpool.tile([C, n_tiles], F32, tag="sq_acc")

            for t in range(n_tiles):
                idx_sb = gpool.tile([128, J], I32, tag="idx")
                nc.sync.dma_start(idx_sb[:], idx_arr[t, :, :])

                g_sb = gpool.tile([128, J, C], F32, tag="g")
                for j in range(J):
                    nc.gpsimd.indirect_dma_start(
                        out=g_sb[:, j, :],
                        out_offset=None,
                        in_=feats[:],
                        in_offset=bass.IndirectOffsetOnAxis(
                            ap=idx_sb[:, j:j + 1], axis=0),
                    )

                out_ps = psA.tile([C, TILE_V], F32, tag="outp")
                for k in range(KVOL):
                    tp = psB.tile([C, TILE_V], F32, tag="tp")
                    for b in range(BLOCKS):
                        nc.tensor.transpose(
                            tp[:, b * 128:(b + 1) * 128],
                            g_sb[:, k * BLOCKS + b, :],
                            ident[:],
                        )
                    rhs = rpool.tile([C, TILE_V], F32R, tag="rhs")
                    nc.vector.tensor_copy(rhs[:], tp[:])
                    nc.tensor.matmul(
                        out_ps[:], w_sbr[:, k, :], rhs[:],
                        start=(k == 0), stop=(k == KVOL - 1),
                    )

                conv_sb = spool.tile([C, TILE_V], F32, tag="conv")
                nc.scalar.activation(
                    conv_sb[:], out_ps[:], AF.Identity,
                    accum_out=sum_acc[:, t:t + 1])
                sq_sb = spool.tile([C, TILE_V], F32, tag="sq")
                nc.scalar.activation(
                    sq_sb[:], conv_sb[:], AF.Square,
                    accum_out=sq_acc[:, t:t + 1])
                nc.sync.dma_start(
                    convT[:, t * TILE_V:(t + 1) * TILE_V], conv_sb[:])

            stats_sb = cpool.tile([C, 2], F32, tag="stats_sb")
            nc.vector.reduce_sum(
                stats_sb[:, 0:1], sum_acc[:], axis=mybir.AxisListType.X)
            nc.vector.reduce_sum(
                stats_sb[:, 1:2], sq_acc[:], axis=mybir.AxisListType.X)
            nc.sync.dma_start(stats[:], stats_sb[:])

    nc.finalize()
    return nc


def build_pass2(n_tiles=N_TILES):
    nc = bacc.Bacc("TRN2", target_bir_lowering=False, debug=False,
                   num_devices=N_CORES)
    convT = nc.dram_tensor("convT", [C, n_tiles * TILE_V], F32,
                           kind="ExternalInput")
    scale = nc.dram_tensor("scale", [C, 1], F32, kind="ExternalInput")
    shift = nc.dram_tensor("shift", [C, 1], F32, kind="ExternalInput")
    out = nc.dram_tensor("out", [n_tiles * TILE_V, C], F32,
                         kind="ExternalOutput")

    with tile.TileContext(nc) as tc:
        with (
            tc.tile_pool(name="const", bufs=1) as cpool,
            tc.tile_pool(name="sp", bufs=3) as spool,
            tc.tile_pool(name="ps", bufs=2, space="PSUM") as ps,
        ):
            ident = cpool.tile([C, C], F32, tag="ident")
            make_identity(nc, ident[:])
            scale_sb = cpool.tile([C, 1], F32, tag="scale")
            shift_sb = cpool.tile([C, 1], F32, tag="shift")
            nc.sync.dma_start(scale_sb[:], scale[:])
            nc.sync.dma_start(shift_sb[:], shift[:])

            for t in range(n_tiles):
                nsb = spool.tile([C, TILE_V], F32, tag="nsb")
                nc.sync.dma_start(
                    nsb[:], convT[:, t * TILE_V:(t + 1) * TILE_V])
                nrm = spool.tile([C, TILE_V], F32, tag="nrm")
                nc.scalar.activation(
                    nrm[:], nsb[:], AF.Relu,
                    bias=shift_sb[:], scale=scale_sb[:])
                op = ps.tile([128, BLOCKS * C], F32, tag="op")
                for b in range(BLOCKS):
                    nc.tensor.transpose(
                        op[:, b * C:(b + 1) * C],
                        nrm[:, b * 128:(b + 1) * 128],
                        ident[:],
                    )
                osb = spool.tile([128, BLOCKS * C], F32, tag="osb")
                nc.vector.tensor_copy(osb[:], op[:])
                nc.sync.dma_start(
                    out[t * TILE_V:(t + 1) * TILE_V, :].rearrange(
                        "(b p) c -> p b c", p=128),
                    osb[:].rearrange("p (b c) -> p b c", b=BLOCKS),
                )

    nc.finalize()
    return nc


# --------------------------------------------------------------------------
# reusable PJRT runner (keeps the compiled executable across calls)
# --------------------------------------------------------------------------
class _Runner:
    """Runs a bass NEFF over n_cores devices via shard_map.

    `replicated`: input names fed once (same array on every core).
    Inputs/outputs are jax arrays; sharded inputs are globally concatenated
    on axis 0 (core-major). Outputs stay on device until converted.
    """

    def __init__(self, nc, n_cores, replicated=()):
        import jax
        from jax.sharding import Mesh, PartitionSpec
        from jax.experimental.shard_map import shard_map
        from concourse import bass2jax

        bass2jax.install_neuronx_cc_hook()
        self.jax = jax
        self.n_cores = n_cores
        self.replicated = set(replicated)
        pname = nc.partition_id_tensor.name if nc.partition_id_tensor else None
        in_names, out_names, out_avals, zero_outs = [], [], [], []
        for alloc in nc.m.functions[0].allocations:
            if not isinstance(alloc, mybir.MemoryLocationSet):
                continue
            name = alloc.memorylocations[0].name
            if alloc.kind == "ExternalInput":
                if name != pname:
                    in_names.append(name)
            elif alloc.kind == "ExternalOutput":
                out_names.append(name)
                shape = tuple(alloc.tensor_shape)
                dtype = mybir.dt.np(alloc.dtype)
                out_avals.append(jax.core.ShapedArray(shape, dtype))
                zero_outs.append(np.zeros(shape, dtype))
        self.in_names, self.out_names = in_names, out_names
        self.out_avals, self.zero_outs = out_avals, zero_outs
        n_params = len(in_names)
        self.n_params = n_params
        all_in = list(in_names) + list(out_names)
        if pname is not None:
            all_in.append(pname)

        def _body(*args):
            operands = list(args)
            if pname is not None:
                operands.append(bass2jax.partition_id_tensor())
            outs = bass2jax._bass_exec_p.bind(
                *operands,
                out_avals=tuple(out_avals),
                in_names=tuple(all_in),
                out_names=tuple(out_names),
                lowering_input_output_aliases=(),
                sim_require_finite=True,
                sim_require_nnan=True,
                nc=nc,
            )
            return tuple(outs)

        devices = jax.devices()[:n_cores]
        mesh = Mesh(np.asarray(devices), ("core",))
        in_specs = tuple(
            PartitionSpec() if n in self.replicated else PartitionSpec("core")
            for n in in_names
        ) + (PartitionSpec("core"),) * len(out_names)
        self.fn = jax.jit(
            shard_map(_body, mesh=mesh, in_specs=in_specs,
                      out_specs=(PartitionSpec("core"),) * len(out_names),
                      check_rep=False),
            keep_unused=True,
        )

    def prep(self, in_map):
        """in_map: replicated name -> array; sharded name -> list of per-core
        arrays OR pre-concatenated global array / jax array."""
        args = []
        for n in self.in_names:
            v = in_map[n]
            if isinstance(v, list):
                v = np.concatenate([np.asarray(x) for x in v], axis=0)
            args.append(v)
        args += [
            np.zeros((self.n_cores * z.shape[0], *z.shape[1:]), z.dtype)
            for z in self.zero_outs
        ]
        return args

    def run(self, in_map):
        outs = self.fn(*self.prep(in_map))
        self.jax.block_until_ready(outs)
        return dict(zip(self.out_names, outs))

    def percore(self, arr_global, name):
        i = self.out_names.index(name)
        return np.asarray(arr_global).reshape(
            self.n_cores, *self.out_avals[i].shape)


# --------------------------------------------------------------------------
# host-side glue
# --------------------------------------------------------------------------
def _arrange_idx(nbr_shard: np.ndarray, n_tiles: int) -> np.ndarray:
    """[n_tiles*512, 27] -> [n_tiles, 128, 108] with
    idx_arr[t, p, k*4+b] = nbr[t*512 + b*128 + p, k]."""
    a = nbr_shard.reshape(n_tiles, BLOCKS, 128, KVOL)       # [t, b, p, k]
    a = a.transpose(0, 2, 3, 1)                             # [t, p, k, b]
    return np.ascontiguousarray(a.reshape(n_tiles, 128, J), dtype=np.int32)


def run_pipeline(feats, nbr, W, gamma, beta, n_tiles):
    n_total = n_tiles * TILE_V * N_CORES
    npc = n_tiles * TILE_V
    key = ("p1", n_tiles)
    if key not in _cache:
        _cache[key] = _Runner(build_pass1(n_tiles), N_CORES,
                              replicated=("feats", "W"))
    r1 = _cache[key]
    key2 = ("p2", n_tiles)
    if key2 not in _cache:
        _cache[key2] = _Runner(build_pass2(n_tiles), N_CORES,
                               replicated=("scale", "shift"))
    r2 = _cache[key2]

    idx_all = np.concatenate([
        _arrange_idx(nbr[c * npc:(c + 1) * npc], n_tiles)
        for c in range(N_CORES)
    ], axis=0)
    res1 = r1.run({"feats": feats, "idx_arr": idx_all, "W": W})

    # combine BN stats on host (768 B per core)
    stats = r1.percore(res1["stats"], "stats")         # [8, 96, 2]
    s = stats.sum(axis=0, dtype=np.float64)
    mean = s[:, 0] / n_total
    var = s[:, 1] / n_total - mean * mean
    inv = 1.0 / np.sqrt(var + BN_EPS)
    scale = (gamma.astype(np.float64).reshape(C) * inv).astype(np.float32)
    shift = (beta.astype(np.float64).reshape(C)
             - mean * gamma.astype(np.float64).reshape(C) * inv
             ).astype(np.float32)

    # convT stays device-resident (jax array) between the passes
    res2 = r2.run({
        "convT": res1["convT"],
        "scale": scale.reshape(C, 1),
        "shift": shift.reshape(C, 1),
    })
    return np.asarray(res2["out"])


def kernel(feats, nbr_idx, W, gamma, beta):
    feats = np.ascontiguousarray(feats, dtype=np.float32)
    W = np.ascontiguousarray(W, dtype=np.float32)
    nbr = np.asarray(nbr_idx)
    gamma = np.asarray(gamma, dtype=np.float32)
    beta = np.asarray(beta, dtype=np.float32)
    return run_pipeline(feats, nbr, W, gamma, beta, N_TILES)


def measure_exec(feats, nbr_idx, W, n_tiles=N_TILES):
    """Re-execute both compiled passes with device-resident inputs and
    return (pass1_s, pass2_s) minimum wall times — an upper bound on device
    exec time (includes one axon dispatch round-trip each)."""
    import time
    import jax

    r1 = _cache[("p1", n_tiles)]
    r2 = _cache[("p2", n_tiles)]
    npc = n_tiles * TILE_V
    nbr = np.asarray(nbr_idx)
    idx_all = np.concatenate([
        _arrange_idx(nbr[c * npc:(c + 1) * npc], n_tiles)
        for c in range(N_CORES)
    ], axis=0)
    args1 = [jax.device_put(a) for a in r1.prep({
        "feats": np.ascontiguousarray(feats, np.float32),
        "idx_arr": idx_all,
        "W": np.ascontiguousarray(W, np.float32)})]
    jax.block_until_ready(args1)
    t1 = []
    for _ in range(4):
        t0 = time.perf_counter()
        outs = r1.fn(*args1)
        jax.block_until_ready(outs)
        t1.append(time.perf_counter() - t0)
    res1 = dict(zip(r1.out_names, outs))
    args2 = r2.prep({
        "convT": res1["convT"],
        "scale": np.ones((C, 1), np.float32),
        "shift": np.zeros((C, 1), np.float32)})
    args2 = [jax.device_put(a) if isinstance(a, np.ndarray) else a
             for a in args2]
    jax.block_until_ready(args2)
    t2 = []
    for _ in range(4):
        t0 = time.perf_counter()
        outs2 = r2.fn(*args2)
        jax.block_until_ready(outs2)
        t2.append(time.perf_counter() - t0)
    return min(t1), min(t2)


# revision 15
# speedup vs baseline: 185.9828x; 1.0427x over previous
"""Trainium2 Bass kernel for nn_BasicConvolutionBlock (sparse conv + BN + ReLU).

Math (per reference):
    conv[n] = sum_k feats[nbr_idx[n, k]] @ W[k]       # [N, 96], k = 0..26
    y = (conv - mean) * rsqrt(var + eps) * gamma + beta ; relu(y)

Distribution: voxel (N) dimension sharded across 8 NeuronCores; feats table
and weights replicated per core.

Two NEFFs (the AllReduce-in-kernel path is unstable under the axon PJRT
bridge, so per-core BN partial sums are combined on the host -- 768 B of
float math -- between the two device passes):

  pass 1 (per core, per 512-voxel tile):
    - 108 indirect DMA gathers (128 rows x 96 f32) -> SBUF [128, 108, 96]
    - per offset k: 4 PE transposes [128v, 96c] -> PSUM [96, 512v]
      DVE copy PSUM -> SBUF rounding to f32r
      accumulating f32r matmul W_k.T @ G_k.T -> PSUM [96, 512]
    - BN partial sum/sumsq via ACT accum; conv staged to DRAM channel-major
  pass 2: y = relu(conv * scale + shift) (fused ACT op), PE transpose back
    to row-major, store.
"""
import numpy as np

import concourse.bass as bass
import concourse.bacc as bacc
import concourse.tile as tile
import concourse.mybir as mybir
from concourse.masks import make_identity

F32 = mybir.dt.float32
F32R = mybir.dt.float32r
I32 = mybir.dt.int32
AF = mybir.ActivationFunctionType

N_TOTAL = 262144
C = 96
KVOL = 27
N_CORES = 8
N_PER_CORE = N_TOTAL // N_CORES      # 32768
TILE_V = 512                         # voxels per tile
BLOCKS = TILE_V // 128               # 4
J = KVOL * BLOCKS                    # 108 gathers per tile
N_TILES = N_PER_CORE // TILE_V       # 64
BN_EPS = 1e-5

_cache = {}


# --------------------------------------------------------------------------
# graph builders
# --------------------------------------------------------------------------
def build_pass1(n_tiles=N_TILES, n_cores=N_CORES, gbufs=2, rbufs=3):
    nc = bacc.Bacc("TRN2", target_bir_lowering=False, debug=False,
                   num_devices=n_cores)

    feats = nc.dram_tensor("feats", [N_TOTAL, C], F32, kind="ExternalInput")
    # host-arranged: idx_arr[t, p, k*4+b] = nbr[t*512 + b*128 + p, k]
    idx_arr = nc.dram_tensor("idx_arr", [n_tiles, 128, J], I32,
                             kind="ExternalInput")
    W = nc.dram_tensor("W", [KVOL, C, C], F32, kind="ExternalInput")
    convT = nc.dram_tensor("convT", [C, n_tiles * TILE_V], F32,
                           kind="ExternalOutput")
    stats = nc.dram_tensor("stats", [C, 2], F32, kind="ExternalOutput")

    with tile.TileContext(nc) as tc:
        with (
            tc.tile_pool(name="const", bufs=1) as cpool,
            tc.tile_pool(name="gp", bufs=gbufs) as gpool,
            tc.tile_pool(name="rp", bufs=rbufs) as rpool,
            tc.tile_pool(name="sp", bufs=2) as spool,
            tc.tile_pool(name="psA", bufs=2, space="PSUM") as psA,
            tc.tile_pool(name="psB", bufs=2, space="PSUM") as psB,
        ):
            ident = cpool.tile([128, 128], F32, tag="ident")
            make_identity(nc, ident[:])
            w_sb = cpool.tile([C, KVOL, C], F32, tag="w")
            nc.sync.dma_start(w_sb[:], W[:].rearrange("k ci co -> ci k co"))
            w_sbr = cpool.tile([C, KVOL, C], F32R, tag="wr")
            nc.vector.tensor_copy(w_sbr[:], w_sb[:])

            sum_acc = cpool.tile([C, n_tiles], F32, tag="sum_acc")
            sq_acc = cpool.tile([C, n_tiles], F32, tag="sq_acc")

            for t in range(n_tiles):
                idx_sb = gpool.tile([128, J], I32, tag="idx")
                nc.sync.dma_start(idx_sb[:], idx_arr[t, :, :])

                g_sb = gpool.tile([128, J, C], F32, tag="g")
                for j in range(J):
                    nc.gpsimd.indirect_dma_start(
                        out=g_sb[:, j, :],
                        out_offset=None,
                        in_=feats[:],
                        in_offset=bass.IndirectOffsetOnAxis(
                            ap=idx_sb[:, j:j + 1], axis=0),
                    )

                out_ps = psA.tile([C, TILE_V], F32, tag="outp")
                for k in range(KVOL):
                    tp = psB.tile([C, TILE_V], F32, tag="tp")
                    for b in range(BLOCKS):
                        nc.tensor.transpose(
                            tp[:, b * 128:(b + 1) * 128],
                            g_sb[:, k * BLOCKS + b, :],
                            ident[:],
                        )
                    rhs = rpool.tile([C, TILE_V], F32R, tag="rhs")
                    nc.vector.tensor_copy(rhs[:], tp[:])
                    nc.tensor.matmul(
                        out_ps[:], w_sbr[:, k, :], rhs[:],
                        start=(k == 0), stop=(k == KVOL - 1),
                    )

                conv_sb = spool.tile([C, TILE_V], F32, tag="conv")
                nc.scalar.activation(
                    conv_sb[:], out_ps[:], AF.Identity,
                    accum_out=sum_acc[:, t:t + 1])
                sq_sb = spool.tile([C, TILE_V], F32, tag="sq")
                nc.scalar.activation(
                    sq_sb[:], conv_sb[:], AF.Square,
                    accum_out=sq_acc[:, t:t + 1])
                nc.sync.dma_start(
                    convT[:, t * TILE_V:(t + 1) * TILE_V], conv_sb[:])

            stats_sb = cpool.tile([C, 2], F32, tag="stats_sb")
            nc.vector.reduce_sum(
                stats_sb[:, 0:1], sum_acc[:], axis=mybir.AxisListType.X)
            nc.vector.reduce_sum(
                stats_sb[:, 1:2], sq_acc[:], axis=mybir.AxisListType.X)
            nc.sync.dma_start(stats[:], stats_sb[:])

    nc.finalize()
    return nc


def build_pass2(n_tiles=N_TILES):
    nc = bacc.Bacc("TRN2", target_bir_lowering=False, debug=False,
                   num_devices=N_CORES)
    convT = nc.dram_tensor("convT", [C, n_tiles * TILE_V], F32,
                           kind="ExternalInput")
    scale = nc.dram_tensor("scale", [C, 1], F32, kind="ExternalInput")
    shift = nc.dram_tensor("shift", [C, 1], F32, kind="ExternalInput")
    out = nc.dram_tensor("out", [n_tiles * TILE_V, C], F32,
                         kind="ExternalOutput")

    with tile.TileContext(nc) as tc:
        with (
            tc.tile_pool(name="const", bufs=1) as cpool,
            tc.tile_pool(name="sp", bufs=3) as spool,
            tc.tile_pool(name="ps", bufs=2, space="PSUM") as ps,
        ):
            ident = cpool.tile([C, C], F32, tag="ident")
            make_identity(nc, ident[:])
            scale_sb = cpool.tile([C, 1], F32, tag="scale")
            shift_sb = cpool.tile([C, 1], F32, tag="shift")
            nc.sync.dma_start(scale_sb[:], scale[:])
            nc.sync.dma_start(shift_sb[:], shift[:])

            for t in range(n_tiles):
                nsb = spool.tile([C, TILE_V], F32, tag="nsb")
                nc.sync.dma_start(
                    nsb[:], convT[:, t * TILE_V:(t + 1) * TILE_V])
                nrm = spool.tile([C, TILE_V], F32, tag="nrm")
                nc.scalar.activation(
                    nrm[:], nsb[:], AF.Relu,
                    bias=shift_sb[:], scale=scale_sb[:])
                op = ps.tile([128, BLOCKS * C], F32, tag="op")
                for b in range(BLOCKS):
                    nc.tensor.transpose(
                        op[:, b * C:(b + 1) * C],
                        nrm[:, b * 128:(b + 1) * 128],
                        ident[:],
                    )
                osb = spool.tile([128, BLOCKS * C], F32, tag="osb")
                nc.vector.tensor_copy(osb[:], op[:])
                nc.sync.dma_start(
                    out[t * TILE_V:(t + 1) * TILE_V, :].rearrange(
                        "(b p) c -> p b c", p=128),
                    osb[:].rearrange("p (b c) -> p b c", b=BLOCKS),
                )

    nc.finalize()
    return nc


# --------------------------------------------------------------------------
# reusable PJRT runner (keeps the compiled executable across calls)
# --------------------------------------------------------------------------
class _Runner:
    """Runs a bass NEFF over n_cores devices via shard_map.

    `replicated`: input names fed once (same array on every core).
    Inputs/outputs are jax arrays; sharded inputs are globally concatenated
    on axis 0 (core-major). Outputs stay on device until converted.
    """

    def __init__(self, nc, n_cores, replicated=()):
        import jax
        from jax.sharding import Mesh, PartitionSpec
        from jax.experimental.shard_map import shard_map
        from concourse import bass2jax

        bass2jax.install_neuronx_cc_hook()
        self.jax = jax
        self.n_cores = n_cores
        self.replicated = set(replicated)
        pname = nc.partition_id_tensor.name if nc.partition_id_tensor else None
        in_names, out_names, out_avals, zero_outs = [], [], [], []
        for alloc in nc.m.functions[0].allocations:
            if not isinstance(alloc, mybir.MemoryLocationSet):
                continue
            name = alloc.memorylocations[0].name
            if alloc.kind == "ExternalInput":
                if name != pname:
                    in_names.append(name)
            elif alloc.kind == "ExternalOutput":
                out_names.append(name)
                shape = tuple(alloc.tensor_shape)
                dtype = mybir.dt.np(alloc.dtype)
                out_avals.append(jax.core.ShapedArray(shape, dtype))
                zero_outs.append(np.zeros(shape, dtype))
        self.in_names, self.out_names = in_names, out_names
        self.out_avals, self.zero_outs = out_avals, zero_outs
        n_params = len(in_names)
        self.n_params = n_params
        all_in = list(in_names) + list(out_names)
        if pname is not None:
            all_in.append(pname)

        def _body(*args):
            operands = list(args)
            if pname is not None:
                operands.append(bass2jax.partition_id_tensor())
            outs = bass2jax._bass_exec_p.bind(
                *operands,
                out_avals=tuple(out_avals),
                in_names=tuple(all_in),
                out_names=tuple(out_names),
                lowering_input_output_aliases=(),
                sim_require_finite=True,
                sim_require_nnan=True,
                nc=nc,
            )
            return tuple(outs)

        devices = jax.devices()[:n_cores]
        mesh = Mesh(np.asarray(devices), ("core",))
        in_specs = tuple(
            PartitionSpec() if n in self.replicated else PartitionSpec("core")
            for n in in_names
        ) + (PartitionSpec("core"),) * len(out_names)
        self.fn = jax.jit(
            shard_map(_body, mesh=mesh, in_specs=in_specs,
                      out_specs=(PartitionSpec("core"),) * len(out_names),
                      check_rep=False),
            keep_unused=True,
        )

    def prep(self, in_map):
        """in_map: replicated name -> array; sharded name -> list of per-core
        arrays OR pre-concatenated global array / jax array."""
        args = []
        for n in self.in_names:
            v = in_map[n]
            if isinstance(v, list):
                v = np.concatenate([np.asarray(x) for x in v], axis=0)
            args.append(v)
        args += [
            np.zeros((self.n_cores * z.shape[0], *z.shape[1:]), z.dtype)
            for z in self.zero_outs
        ]
        return args

    def run(self, in_map):
        outs = self.fn(*self.prep(in_map))
        self.jax.block_until_ready(outs)
        return dict(zip(self.out_names, outs))

    def percore(self, arr_global, name):
        i = self.out_names.index(name)
        return np.asarray(arr_global).reshape(
            self.n_cores, *self.out_avals[i].shape)


# --------------------------------------------------------------------------
# host-side glue
# --------------------------------------------------------------------------
def _arrange_idx(nbr_shard: np.ndarray, n_tiles: int) -> np.ndarray:
    """[n_tiles*512, 27] -> [n_tiles, 128, 108] with
    idx_arr[t, p, k*4+b] = nbr[t*512 + b*128 + p, k]."""
    a = nbr_shard.reshape(n_tiles, BLOCKS, 128, KVOL)       # [t, b, p, k]
    a = a.transpose(0, 2, 3, 1)                             # [t, p, k, b]
    return np.ascontiguousarray(a.reshape(n_tiles, 128, J), dtype=np.int32)


def run_pipeline(feats, nbr, W, gamma, beta, n_tiles):
    n_total = n_tiles * TILE_V * N_CORES
    npc = n_tiles * TILE_V
    key = ("p1", n_tiles)
    if key not in _cache:
        _cache[key] = _Runner(build_pass1(n_tiles), N_CORES,
                              replicated=("feats", "W"))
    r1 = _cache[key]
    key2 = ("p2", n_tiles)
    if key2 not in _cache:
        _cache[key2] = _Runner(build_pass2(n_tiles), N_CORES,
                               replicated=("scale", "shift"))
    r2 = _cache[key2]

    idx_all = np.concatenate([
        _arrange_idx(nbr[c * npc:(c + 1) * npc], n_tiles)
        for c in range(N_CORES)
    ], axis=0)
    res1 = r1.run({"feats": feats, "idx_arr": idx_all, "W": W})

    # combine BN stats on host (768 B per core)
    stats = r1.percore(res1["stats"], "stats")         # [8, 96, 2]
    s = stats.sum(axis=0, dtype=np.float64)
    mean = s[:, 0] / n_total
    var = s[:, 1] / n_total - mean * mean
    inv = 1.0 / np.sqrt(var + BN_EPS)
    scale = (gamma.astype(np.float64).reshape(C) * inv).astype(np.float32)
    shift = (beta.astype(np.float64).reshape(C)
             - mean * gamma.astype(np.float64).reshape(C) * inv
             ).astype(np.float32)

    # convT stays device-resident (jax array) between the passes
    res2 = r2.run({
        "convT": res1["convT"],
        "scale": scale.reshape(C, 1),
        "shift": shift.reshape(C, 1),
    })
    return np.asarray(res2["out"])


def kernel(feats, nbr_idx, W, gamma, beta):
    feats = np.ascontiguousarray(feats, dtype=np.float32)
    W = np.ascontiguousarray(W, dtype=np.float32)
    nbr = np.asarray(nbr_idx)
    gamma = np.asarray(gamma, dtype=np.float32)
    beta = np.asarray(beta, dtype=np.float32)
    return run_pipeline(feats, nbr, W, gamma, beta, N_TILES)


def measure_exec(feats, nbr_idx, W, n_tiles=N_TILES):
    """Re-execute both compiled passes with device-resident inputs and
    return (pass1_s, pass2_s) minimum wall times — an upper bound on device
    exec time (includes one axon dispatch round-trip each)."""
    import time
    import jax

    r1 = _cache[("p1", n_tiles)]
    r2 = _cache[("p2", n_tiles)]
    npc = n_tiles * TILE_V
    nbr = np.asarray(nbr_idx)
    idx_all = np.concatenate([
        _arrange_idx(nbr[c * npc:(c + 1) * npc], n_tiles)
        for c in range(N_CORES)
    ], axis=0)
    args1 = [jax.device_put(a) for a in r1.prep({
        "feats": np.ascontiguousarray(feats, np.float32),
        "idx_arr": idx_all,
        "W": np.ascontiguousarray(W, np.float32)})]
    jax.block_until_ready(args1)
    t1 = []
    for _ in range(4):
        t0 = time.perf_counter()
        outs = r1.fn(*args1)
        jax.block_until_ready(outs)
        t1.append(time.perf_counter() - t0)
    res1 = dict(zip(r1.out_names, outs))
    args2 = r2.prep({
        "convT": res1["convT"],
        "scale": np.ones((C, 1), np.float32),
        "shift": np.zeros((C, 1), np.float32)})
    args2 = [jax.device_put(a) if isinstance(a, np.ndarray) else a
             for a in args2]
    jax.block_until_ready(args2)
    t2 = []
    for _ in range(4):
        t0 = time.perf_counter()
        outs2 = r2.fn(*args2)
        jax.block_until_ready(outs2)
        t2.append(time.perf_counter() - t0)
    return min(t1), min(t2)
